# revision 1
# baseline (speedup 1.0000x reference)
"""LIF spiking network forward (nn_LIFSG) on 8 Trainium2 NeuronCores.

Math (per reference):
    I = einsum('bti,oi->bto', spikes, W)         # GEMM
    u_t = decay * v_{t-1} + I_t                  # leaky integrate
    s_t = (u_t - 1 > 0)                          # spike
    v_t = u_t * (1 - s_t)                        # reset to zero

Sharding: data-parallel over B (32 batches -> 4 per core). Each core:
  - GEMM as I[o, t] = (W^T tile).T @ (spikes^T tile) on the PE array.
    W is split into 3 bf16 terms (hi/mid/lo); spikes are binary so every
    bf16 product is exact -> fp32-exact GEMM in PSUM. Weight tiles are
    reused across the 4 batches (8 PSUM banks held) to hide LDWEIGHTS.
  - The T=1000 recurrence runs as 1000 fused custom-DVE instructions
    (one LIF step per instruction over all 2048 lanes of the core):
        u_new = select(u_old < nextafter(1), u_old, 0) * decay + I_t
    This is the critical path (~145 ns/dependent step on HW), so time
    chunks are graduated (128/372/500) to start the chain early, and
    spike extraction is moved to the Scalar engine.
  - Spikes extracted on ACT: s = relu(sign(u - 1))  (exact {0,1}),
    streamed out as [b, o, t]; the host transposes back to (B, T, n_out)
    during unshard.

Host-side work is limited to sharding/layout prep (transpose + dtype
cast + W splitting) and the inverse gather; all FLOPs run on device.
"""

import sys

sys.path.insert(0, "/opt/trn_rl_repo")

import numpy as np
import ml_dtypes

import concourse.bacc as bacc
import concourse.tile as tile
import concourse.mybir as mybir
import concourse.dve_ops as dve_ops
from concourse.dve_ops import DveOp
from concourse.dve_spec import C0, C1, Spec, Src0, Src1, Zero, lower, select
from concourse.dve_uop import DveOpSpec
from concourse.bass_utils import run_bass_kernel_spmd

# ---------------- problem constants (hardcoded from spec) ----------------
B, T, N_IN, N_OUT = 32, 1000, 1024, 512
N_CORES = 8
B_SH = B // N_CORES          # 4 batches per core
DECAY = float(np.exp(-1.0 / 20.0))
# u < nextafter(1.0)  <=>  u <= 1.0 in fp32
THRESH_LT = float(np.nextafter(np.float32(1.0), np.float32(np.inf)))

CH_LIST = [128, 372, 500]    # graduated: small head starts the chain early (HW-measured best order)
CH_MAX = max(CH_LIST)
N_IT = N_IN // 128           # 8 contraction tiles
N_OT = N_OUT // 128          # 4 output-partition tiles
LANES = B_SH * N_OT          # 16 scan lanes per core (free dim)
N_SPLIT = 3                  # bf16 splits of W

AF = mybir.ActivationFunctionType


# ---------------- custom DVE op: one LIF step per instruction ----------------
def _lif_ref(in0, in1, c0, c1, c2):
    y = np.where(in0.astype(np.float32) < c1, in0, 0.0).astype(np.float32)
    return (y * np.float32(c0) + in1.astype(np.float32)).astype(np.float32)


_LIF_SPEC = Spec(body=select(Src0 < C1, Src0, Zero) * C0 + Src1, reference=_lif_ref)
_LIF_NAME = "LIF_STEP_ANT"


def _register_lif_op() -> DveOp:
    if _LIF_NAME in dve_ops._SUB_OPCODE_FOR_NAME:
        for op in dve_ops.OPS:
            if op.name == _LIF_NAME:
                return op
    opcode = dve_ops._CUSTOM_DVE_ROW_BASE + len(dve_ops.OPS)
    assert opcode < 0x20
    dve_ops._SUB_OPCODE_FOR_NAME[_LIF_NAME] = opcode
    shas = {}
    for ver in ("v3", "v4"):
        tmp = DveOpSpec(
            name=_LIF_NAME, opcode=opcode, uops=lower(_LIF_SPEC, ver=ver), rd1_en=True
        )
        shas[ver] = tmp.sha(ver)
    op = DveOp(_LIF_NAME, _LIF_SPEC, subdim=False, uops_sha=shas)
    dve_ops.OPS.append(op)
    dve_ops.CUSTOM_DVE_SPECS[_LIF_NAME] = _LIF_SPEC
    return op


# ---------------- device kernel ----------------
def _build_kernel():
    LIF = _register_lif_op()
    nc = bacc.Bacc("TRN2", target_bir_lowering=False, debug=False, num_devices=N_CORES)
    # Register a -1.0 const AP (used as the ACT Sign bias); mirrors the
    # built-in const registration in Bass.__init__.
    _cm1 = nc.alloc_sbuf_tensor("const-float32-neg1", [128, 1], mybir.dt.float32)
    nc.gpsimd.memset(_cm1.ap(), -1.0)
    nc.const_aps.aps[(mybir.dt.float32, -1.0)] = _cm1.ap()
    nc.all_engine_barrier()
    xT = nc.dram_tensor("xT", [B_SH, N_IN, T], mybir.dt.bfloat16, kind="ExternalInput")
    wts = nc.dram_tensor(
        "wts", [N_SPLIT, N_IN, N_OUT], mybir.dt.bfloat16, kind="ExternalInput"
    )
    out = nc.dram_tensor("out", [B_SH, N_OUT, T], mybir.dt.float32, kind="ExternalOutput")

    with tile.TileContext(nc) as tc:
        with (
            tc.tile_pool(name="wx", bufs=1) as wx_pool,
            tc.tile_pool(name="state", bufs=1) as state_pool,
            tc.tile_pool(name="spk", bufs=4) as spk_pool,
            tc.tile_pool(name="mm", bufs=8, space="PSUM") as psum_pool,
        ):
            # Stationary weights: [128p, split, it, o]. One DMA per split so
            # the s=0 matmuls only wait for the first 1MB.
            w_sb = wx_pool.tile([128, N_SPLIT, N_IT, N_OUT], mybir.dt.bfloat16, tag="w")
            wts_r = wts.rearrange("s (it p) o -> p s it o", p=128)
            for s in range(N_SPLIT):
                nc.sync.dma_start(w_sb[:, s], wts_r[:, s])
            # Spike inputs, all resident: one tile per batch [128p, it, T].
            # Split each DMA into head (chunk-0 columns) + rest so the first
            # chunk's matmuls start after ~1MB instead of 8MB of input DMA.
            head = CH_LIST[0]
            x_sb = []
            for b in range(B_SH):
                xt = wx_pool.tile(
                    [128, N_IT, T], mybir.dt.bfloat16, tag=f"x{b}", name=f"x{b}"
                )
                xTb = xT[b].rearrange("(it p) t -> p it t", p=128)
                nc.sync.dma_start(xt[:, :, :head], xTb[:, :, :head])
                nc.sync.dma_start(xt[:, :, head:], xTb[:, :, head:])
                x_sb.append(xt)

            # Scan state (u trajectory) and GEMM output, ping-pong per chunk
            # Per-parity max chunk length (ping-pong buffers sized to need)
            chmax = [max(c for i, c in enumerate(CH_LIST) if i % 2 == k) for k in range(2)]
            U = [
                state_pool.tile(
                    [128, LANES, chmax[k] + 1], mybir.dt.float32, tag=f"U{k}", name=f"U{k}"
                )
                for k in range(2)
            ]
            Ibuf = [
                state_pool.tile(
                    [128, LANES, chmax[k]], mybir.dt.float32, tag=f"I{k}", name=f"I{k}"
                )
                for k in range(2)
            ]
            zero_col = state_pool.tile([128, LANES], mybir.dt.float32, tag="z")
            nc.vector.memset(zero_col[:], 0.0)

            t0 = 0
            prev_ch = 0
            for ic, ch in enumerate(CH_LIST):
                pc = ic % 2
                # ---- GEMM for this chunk: I[o, t] per (b, ot) lane ----
                # Weight tile outer, batch inner: each loaded weight feeds 4
                # matmuls; 8 PSUM banks (2 ot x 4 b) accumulate concurrently.
                for half in range(2):
                    ots = (2 * half, 2 * half + 1)
                    pss = {
                        (ot, b): psum_pool.tile(
                            [128, ch], mybir.dt.float32, tag="ps", name="ps"
                        )
                        for ot in ots
                        for b in range(B_SH)
                    }
                    for s in range(N_SPLIT):
                        for it in range(N_IT):
                            for ot in ots:
                                w_ap = w_sb[:, s, it, ot * 128 : (ot + 1) * 128]
                                for b in range(B_SH):
                                    nc.tensor.matmul(
                                        pss[(ot, b)][:],
                                        w_ap,
                                        x_sb[b][:, it, t0 : t0 + ch],
                                        start=(s == 0 and it == 0),
                                        stop=(s == N_SPLIT - 1 and it == N_IT - 1),
                                    )
                    for ot in ots:
                        for b in range(B_SH):
                            lane = b * N_OT + ot
                            nc.scalar.copy(Ibuf[pc][:, lane, :ch], pss[(ot, b)][:])

                # ---- LIF chain: one custom-DVE instruction per timestep ----
                for j in range(ch):
                    if ic == 0 and j == 0:
                        prev = zero_col[:]
                    elif j == 0:
                        prev = U[1 - pc][:, :, prev_ch]
                    else:
                        prev = U[pc][:, :, j]
                    nc.vector._custom_dve(
                        LIF,
                        out=U[pc][:, :, j + 1],
                        in0=prev,
                        in1=Ibuf[pc][:, :, j],
                        s0=DECAY,
                        s1=THRESH_LT,
                    )

                # ---- spike extraction on ACT: s = relu(sign(u - 1)) ----
                # Last chunk: extract in two column-halves so the first half
                # (and its store) overlaps the still-running chain tail.
                pieces = (
                    [(1, ch // 2), (1 + ch // 2, ch - ch // 2)]
                    if ic == len(CH_LIST) - 1
                    else [(1, ch)]
                )
                for c0, clen in pieces:
                    for b in range(B_SH):
                        for ot in range(N_OT):
                            lane = b * N_OT + ot
                            st = spk_pool.tile(
                                [128, CH_MAX], mybir.dt.float32, tag="s", name="s"
                            )
                            nc.vector.tensor_scalar(
                                st[:, :clen],
                                U[pc][:, lane, c0 : c0 + clen],
                                1.0,
                                None,
                                mybir.AluOpType.is_gt,
                            )
                            nc.sync.dma_start(
                                out[
                                    b,
                                    ot * 128 : (ot + 1) * 128,
                                    t0 + c0 - 1 : t0 + c0 - 1 + clen,
                                ],
                                st[:, :clen],
                            )
                t0 += ch
                prev_ch = ch

    _dedupe_ldweights(nc)
    nc.compile()
    return nc


def _dedupe_ldweights(nc):
    """Remove back-to-back redundant Ldweights.

    The batch-inner GEMM loop issues 4 matmuls per weight tile; bass emits
    an Ldweights per matmul, so 3 of every 4 weight loads re-load the array
    with the bits it already holds (~75us of PE time). The PE keeps the
    stationary operand until the next Ldweights, so a duplicate load whose
    weights AP is identical to the previous one is a no-op -- drop it,
    provided it carries no semaphore waits/updates and only Matmult
    instructions sit in between (nothing else can clobber the array, and
    the weight tile in SBUF is written once at kernel start).
    """

    def _key(inst):
        a = inst.ins[0]
        try:
            return (a.memory_location().name, a.offset, str(a.ap))
        except Exception:
            return None

    removed = 0
    for blk in nc.m.functions[0].blocks:
        prev_key = None
        keep = []
        for inst in blk.instructions:
            if inst.opcode == "Ldweights":
                k = _key(inst)
                plain = not inst.sync_info and k is not None
                if plain and k == prev_key:
                    removed += 1
                    continue
                prev_key = k if plain else None
            elif inst.opcode != "Matmult":
                prev_key = None
            keep.append(inst)
        blk.instructions = keep
    return removed


_NC_CACHE = None


def _prep_inputs(input_spikes_seq: np.ndarray, W: np.ndarray):
    W32 = np.ascontiguousarray(np.asarray(W, dtype=np.float32).T)   # [n_in, n_out]
    w_hi = W32.astype(ml_dtypes.bfloat16)
    r1 = W32 - w_hi.astype(np.float32)
    w_mid = r1.astype(ml_dtypes.bfloat16)
    w_lo = (r1 - w_mid.astype(np.float32)).astype(ml_dtypes.bfloat16)
    wts = np.ascontiguousarray(np.stack([w_hi, w_mid, w_lo])[:N_SPLIT])

    x = np.asarray(input_spikes_seq, dtype=np.float32)
    in_maps = []
    for c in range(N_CORES):
        xs = x[c * B_SH : (c + 1) * B_SH]                           # [4, T, n_in]
        xs_T = np.ascontiguousarray(xs.transpose(0, 2, 1)).astype(ml_dtypes.bfloat16)
        in_maps.append({"xT": xs_T, "wts": wts})
    return in_maps


def kernel(input_spikes_seq: np.ndarray, W: np.ndarray) -> np.ndarray:
    global _NC_CACHE
    if _NC_CACHE is None:
        _NC_CACHE = _build_kernel()
    nc = _NC_CACHE

    in_maps = _prep_inputs(input_spikes_seq, W)
    res = run_bass_kernel_spmd(nc, in_maps, core_ids=list(range(N_CORES)))

    # ---- gather/unshard: [core][4, n_out, T] -> (B, T, n_out) ----
    outs = [r["out"] for r in res.results]
    full = np.concatenate(outs, axis=0)                             # [B, n_out, T]
    return np.ascontiguousarray(full.transpose(0, 2, 1))



# revision 4
# speedup vs baseline: 2.0259x; 2.0259x over previous
"""LIF spiking network forward (nn_LIFSG) on 8 Trainium2 NeuronCores.

Math (per reference):
    I = einsum('bti,oi->bto', spikes, W)         # GEMM
    u_t = decay * v_{t-1} + I_t                  # leaky integrate
    s_t = (u_t - 1 > 0)                          # spike
    v_t = u_t * (1 - s_t)                        # reset to zero

Sharding: data-parallel over B (32 batches -> 4 per core).

Per core, two stages pipelined per batch:

1. GEMM on the PE array, 2 fp16 passes (exact to ~2^-22 relative):
   V = W^T * 2^9 is split as V ~= fp16(V) + fp16(V - fp16(V)); spikes are
   binary so every fp16 product is exact, and both splits accumulate into
   the same PSUM bank at the 2^9 scale. The PSUM->SBUF copy on the Scalar
   engine applies the exact 2^-9 rescale. 2 passes instead of the 3
   bf16 passes the naive split needs -> ~27us of PE streaming per batch.

2. The T=1000 recurrence as a *streaming scan* on the Vector engine —
   a hand-written custom-DVE uOp program (one instruction per
   (batch, out-tile) lane, II=2 cycles/element) instead of 1000
   dependent ~145ns instructions. Reformulated in scaled coordinates:
       Q_j = sum_{k<=j} d^{-k} I_k        (ADD-scan, 1-cycle feedback)
       spike_j = C_{j-1} < Q_j - d^{-j}   (threshold in scaled units)
       C_j = spike ? Q_j : C_{j-1}        (reset state; 2-cycle feedback
                                           via the out_a/NEXT_ALU_OUT_A path)
   which is exactly u_j = d^j (Q_j - C_{j-1}), spike iff u_j > 1, v reset
   to 0. The instruction outputs the spike train directly (fp32 0/1),
   so no separate spike-extraction pass and no u-trajectory storage.
   Validated bit-exact offline against the fp32 reference recurrence.

Host-side work is limited to sharding/layout prep (transpose + dtype
cast + W splitting + the d^{-j} row) and the inverse gather.
"""

import sys

sys.path.insert(0, "/opt/trn_rl_repo")

import numpy as np

import concourse.bacc as bacc
import concourse.tile as tile
import concourse.mybir as mybir
import concourse.dve_ops as dve_ops
from concourse.dve_ops import DveOp
from concourse.dve_spec import Spec, Src0, Src1
from concourse.dve_uop import (
    ENABLE,
    AluInp,
    AluOp,
    DelayInp,
    DveOpSpec,
    InpSel,
    OutPath,
    OutSel,
    Trigger,
    UopConfig,
)
from concourse.bass_utils import run_bass_kernel_spmd

# ---------------- problem constants (hardcoded from spec) ----------------
B, T, N_IN, N_OUT = 32, 1000, 1024, 512
N_CORES = 8
B_SH = B // N_CORES          # 4 batches per core
DECAY = float(np.exp(-1.0 / 20.0))
W_SCALE = 9                  # weights carried at 2^9; PSUM copy applies 2^-9
N_SPLIT = 2                  # fp16 splits of W*2^9
N_IT = N_IN // 128           # 8 contraction tiles
N_OT = N_OUT // 128          # 4 output-partition tiles
TC = 2                       # PSUM column chunks per (b, ot) — 500 fp32 = 1 bank
TCH = T // TC


# ---------------- custom DVE op: streaming LIF scan (II=2) ----------------
# Datapath (8 stages, v3):
#   inp0 = SRC_0 (I_j) -> stage-0 ALU port; inp1 = SRC_1 (d^{-j}) -> lane 0;
#   inp2 = ZERO -> lane 1.
#   s0: e = I * drow
#   s1: Q += e                      [first element: Q = e]
#   s2: R = Q - drow; lane2 <- Q
#   s3: b = (C < R)  (C via NEXT_ALU_OUT_A = s4's out_a flop)
#                                   [first element: C = 0 via lane 1]
#   s4: C = SELECT(pred=b; truthy Q, falsy CURR(s4)); out_a; lane3 <- b
#   s5..s7: pass-through; WR0_LO <- DELAY_3 (the spike bit)
# FSM: uop0 STEP(consume,1) -> uop1 BUBBLE(1 cycle) -> uop2 STEADY(consume,1)
# -> uop1 ... ; SRC_TENSOR_DONE -> IDLE. Elements issue every 2nd cycle so
# both feedback paths (same-stage CURR for Q, one-back NEXT_A for C) are ready.

_LANE_DROW, _LANE_ZERO, _LANE_Q, _LANE_B = 0, 1, 2, 3
_SCAN_LANES = (_LANE_DROW, _LANE_ZERO, _LANE_Q, _LANE_B)


def _scan_body_uop(step: bool) -> UopConfig:
    u = UopConfig()
    u.enable_input(InpSel.SRC_0, 0)
    u.enable_input(InpSel.SRC_1, _LANE_DROW + 1)
    u.enable_input(InpSel.ZERO, _LANE_ZERO + 1)
    dp = u.datapath_config
    for st in range(8):
        dp[st].pass_through_delay(*_SCAN_LANES)
    dp[0].enable_alu(AluOp.MULTIPLY, AluInp.PREV_ALU_OUT, AluInp.PREV_DELAY_0)
    if step:
        dp[1].enable_alu(AluOp.BYPASS, AluInp.PREV_ALU_OUT, AluInp.PREV_ALU_OUT)
    else:
        dp[1].enable_alu(AluOp.ADD, AluInp.CURR_ALU_OUT, AluInp.PREV_ALU_OUT)
    dp[2].enable_alu(AluOp.SUBTRACT, AluInp.PREV_ALU_OUT, AluInp.PREV_DELAY_0)
    dp[2].enable_delay_from_src(DelayInp.PREV_ALU_OUT, _LANE_Q)
    c_src = AluInp.PREV_DELAY_1 if step else AluInp.NEXT_ALU_OUT_A
    dp[3].enable_alu(AluOp.IS_LT, c_src, AluInp.PREV_ALU_OUT)
    falsy = AluInp.PREV_DELAY_1 if step else AluInp.CURR_ALU_OUT
    dp[4].enable_alu(AluOp.SELECT, falsy, AluInp.PREV_DELAY_2)
    dp[4].alu_out_a_enable = ENABLE
    dp[4].enable_delay_from_src(DelayInp.PREV_ALU_OUT, _LANE_B)
    u.enable_output(OutSel.DELAY_3, OutPath.WR0_LO)
    u.require_inp0 = ENABLE
    u.require_inp1 = ENABLE
    u.repeat_count = 1
    u.trigger = (Trigger.SRC_TENSOR_DONE, Trigger.COUNT, Trigger.NONE)
    u.next_uop = (0, 1, 0)
    return u


def _scan_bubble_uop() -> UopConfig:
    u = UopConfig()
    u.repeat_count = 1
    u.trigger = (Trigger.COUNT, Trigger.NONE, Trigger.NONE)
    u.next_uop = (2, 0, 0)
    return u


def _lif_scan_uops() -> list[UopConfig]:
    return [_scan_body_uop(step=True), _scan_bubble_uop(), _scan_body_uop(step=False)]


def _lif_scan_reference(in0, in1, c0, c1, c2):
    """CoreSim reference: exact fp32 op order of the datapath.
    in0: [P, T] GEMM currents; in1: [P, T] d^{-j} row. Returns [P, T] spikes."""
    I = np.asarray(in0, np.float32)
    drow = np.broadcast_to(np.asarray(in1, np.float32), I.shape)
    P, T_ = I.shape
    Q = np.zeros(P, np.float32)
    C = np.zeros(P, np.float32)
    out = np.zeros((P, T_), np.float32)
    for j in range(T_):
        e = (I[:, j] * drow[:, j]).astype(np.float32)
        Q = (Q + e).astype(np.float32) if j else e
        R = (Q - drow[:, j]).astype(np.float32)
        b = C < R
        out[:, j] = b
        C = np.where(b, Q, C)
    return out


_SCAN_NAME = "LIF_SCAN_ANT"


class _HandDveOp(DveOp):
    """DveOp whose uOp program is hand-written (bypasses lower())."""

    def compile(self, ver):
        key = (self.name, ver)
        if (r := dve_ops._COMPILE_CACHE.get(key)) is not None:
            return r
        assert ver == "v3", "LIF scan op is TRN2/v3 only"
        spec = DveOpSpec(
            name=self.name,
            opcode=dve_ops.get_dve_sub_opcode(self.name),
            uops=_lif_scan_uops(),
            rd1_en=True,
        )
        spec.validate(ver)
        dve_ops._COMPILE_CACHE[key] = spec
        return spec


def _register_scan_op() -> DveOp:
    if _SCAN_NAME in dve_ops._SUB_OPCODE_FOR_NAME:
        for op in dve_ops.OPS:
            if op.name == _SCAN_NAME:
                return op
    opcode = dve_ops._CUSTOM_DVE_ROW_BASE + len(dve_ops.OPS)
    assert opcode < 0x20
    dve_ops._SUB_OPCODE_FOR_NAME[_SCAN_NAME] = opcode
    spec = Spec(body=Src0 * Src1, reference=_lif_scan_reference)
    op = _HandDveOp(_SCAN_NAME, spec, subdim=False, uops_sha={})
    dve_ops.OPS.append(op)
    dve_ops.CUSTOM_DVE_SPECS[_SCAN_NAME] = spec
    return op


# ---------------- device kernel ----------------
def _build_kernel():
    LIF_SCAN = _register_scan_op()
    nc = bacc.Bacc("TRN2", target_bir_lowering=False, debug=False, num_devices=N_CORES)
    xT = nc.dram_tensor("xT", [B_SH, N_IN, T], mybir.dt.float16, kind="ExternalInput")
    wts = nc.dram_tensor(
        "wts", [N_SPLIT, N_IN, N_OUT], mybir.dt.float16, kind="ExternalInput"
    )
    drow = nc.dram_tensor("drow", [128, T], mybir.dt.float32, kind="ExternalInput")
    out = nc.dram_tensor("out", [B_SH, N_OUT, T], mybir.dt.float32, kind="ExternalOutput")

    with tile.TileContext(nc) as tc:
        with (
            tc.tile_pool(name="wx", bufs=1) as wx_pool,
            tc.tile_pool(name="io", bufs=2) as io_pool,
            tc.tile_pool(name="mm", bufs=8, space="PSUM") as psum_pool,
        ):
            # Stationary weights [128p, split, it, o] and the d^{-j} row.
            w_sb = wx_pool.tile([128, N_SPLIT, N_IT, N_OUT], mybir.dt.float16, tag="w")
            nc.sync.dma_start(w_sb[:], wts.rearrange("s (it p) o -> p s it o", p=128))
            drow_sb = wx_pool.tile([128, T], mybir.dt.float32, tag="drow")
            nc.sync.dma_start(drow_sb[:], drow[:, :])
            # Spike inputs, all resident: one tile per batch [128p, it, T].
            x_sb = []
            for b in range(B_SH):
                xt = wx_pool.tile(
                    [128, N_IT, T], mybir.dt.float16, tag=f"x{b}", name=f"x{b}"
                )
                nc.sync.dma_start(xt[:], xT[b].rearrange("(it p) t -> p it t", p=128))
                x_sb.append(xt)

            for b in range(B_SH):
                Ib = io_pool.tile([128, N_OT, T], mybir.dt.float32, tag="I", name=f"I{b}")
                sp = io_pool.tile([128, N_OT, T], mybir.dt.float32, tag="s", name=f"s{b}")
                for ot in range(N_OT):
                    # ---- GEMM: both fp16 splits accumulate into one bank ----
                    pss = [
                        psum_pool.tile([128, TCH], mybir.dt.float32, tag="ps", name="ps")
                        for _ in range(TC)
                    ]
                    for s in range(N_SPLIT):
                        for it in range(N_IT):
                            w_ap = w_sb[:, s, it, ot * 128 : (ot + 1) * 128]
                            for tcb in range(TC):
                                nc.tensor.matmul(
                                    pss[tcb][:],
                                    w_ap,
                                    x_sb[b][:, it, tcb * TCH : (tcb + 1) * TCH],
                                    start=(s == 0 and it == 0),
                                    stop=(s == N_SPLIT - 1 and it == N_IT - 1),
                                )
                    # PSUM -> SBUF with the exact 2^-9 rescale (Scalar engine).
                    for tcb in range(TC):
                        nc.scalar.mul(
                            Ib[:, ot, tcb * TCH : (tcb + 1) * TCH],
                            pss[tcb][:],
                            2.0 ** -W_SCALE,
                        )
                    # ---- streaming LIF scan: spikes out in one instruction;
                    # issued per-ot so scans overlap the next ot's GEMM ----
                    nc.vector._custom_dve(
                        LIF_SCAN,
                        out=sp[:, ot],
                        in0=Ib[:, ot],
                        in1=drow_sb[:],
                        s0=0.0,
                        s1=0.0,
                    )
                    nc.sync.dma_start(out[b, ot * 128 : (ot + 1) * 128, :], sp[:, ot])

    _dedupe_ldweights(nc)
    nc.compile()
    return nc


def _dedupe_ldweights(nc):
    """Remove back-to-back redundant Ldweights.

    The tc-inner GEMM loop issues 2 matmuls per weight tile; bass emits an
    Ldweights per matmul, so half the weight loads re-load the array with
    the bits it already holds. The PE keeps the stationary operand until
    the next Ldweights, so a duplicate load whose weights AP is identical
    to the previous one is a no-op -- drop it, provided it carries no
    semaphore waits/updates and only Matmult instructions sit in between.
    """

    def _key(inst):
        a = inst.ins[0]
        try:
            return (a.memory_location().name, a.offset, str(a.ap))
        except Exception:
            return None

    removed = 0
    for blk in nc.m.functions[0].blocks:
        prev_key = None
        keep = []
        for inst in blk.instructions:
            if inst.opcode == "Ldweights":
                k = _key(inst)
                plain = not inst.sync_info and k is not None
                if plain and k == prev_key:
                    removed += 1
                    continue
                prev_key = k if plain else None
            elif inst.opcode != "Matmult":
                prev_key = None
            keep.append(inst)
        blk.instructions = keep
    return removed


_NC_CACHE = None


def _prep_inputs(input_spikes_seq: np.ndarray, W: np.ndarray):
    V = np.ascontiguousarray(np.asarray(W, dtype=np.float32).T) * np.float32(
        2.0**W_SCALE
    )                                                            # [n_in, n_out]
    w_hi = V.astype(np.float16)
    w_mid = (V - w_hi.astype(np.float32)).astype(np.float16)
    wts = np.ascontiguousarray(np.stack([w_hi, w_mid]))

    # d^{-j} row, fp32 iterative (exact op order the scan assumes)
    drow_1 = np.zeros(T, np.float32)
    acc = np.float32(1.0)
    inv = np.float32(1.0) / np.float32(DECAY)
    for j in range(T):
        drow_1[j] = acc
        acc = np.float32(acc * inv)
    drow = np.ascontiguousarray(np.broadcast_to(drow_1, (128, T)))

    x = np.asarray(input_spikes_seq, dtype=np.float32)
    in_maps = []
    for c in range(N_CORES):
        xs = x[c * B_SH : (c + 1) * B_SH]                        # [4, T, n_in]
        xs_T = np.ascontiguousarray(xs.transpose(0, 2, 1)).astype(np.float16)
        in_maps.append({"xT": xs_T, "wts": wts, "drow": drow})
    return in_maps


def kernel(input_spikes_seq: np.ndarray, W: np.ndarray) -> np.ndarray:
    global _NC_CACHE
    if _NC_CACHE is None:
        _NC_CACHE = _build_kernel()
    nc = _NC_CACHE

    in_maps = _prep_inputs(input_spikes_seq, W)
    res = run_bass_kernel_spmd(nc, in_maps, core_ids=list(range(N_CORES)))

    # ---- gather/unshard: [core][4, n_out, T] -> (B, T, n_out) ----
    outs = [r["out"] for r in res.results]
    full = np.concatenate(outs, axis=0)                          # [B, n_out, T]
    return np.ascontiguousarray(full.transpose(0, 2, 1))
